# revision 1
# baseline (speedup 1.0000x reference)
"""ConcatCritic all-pairs MLP scores on 8 Trainium2 NeuronCores.

scores[i, j] = MLP(concat(x[j], y[i])) computed as a [B, B] grid, sharded
by y-rows across 8 cores (each core computes a [B/8, B] slab).

Key restructure: layer 1 of the MLP acts on concat(x[j], y[i]), so
    z1[i, j, :] = x[j] @ W1x + (y[i] @ W1y + b1)
which is precomputed once as AT = (x @ W1x).T  [H, B] and
CT = (y_slab @ W1y + b1).T  [H, R].  Per row i, h1.T = relu(AT + CT[:, i])
is a single per-partition scalar add+max on the vector engine. This removes
the [B*B, 256] @ [256, 512] matmul entirely.

Layers 2/3 run on the tensor engine in float32r (FP22 multiplies, FP32
accumulate) at 1 cycle/row -- 4x the speed of true-fp32 matmuls with
~2^-12 relative precision.
"""

import threading

import numpy as np

B = 512
DX = 128
DY = 128
H = 512
P = 128
NCORES = 8
R = B // NCORES  # 64 rows of the pair grid per core
HB = H // P  # 4 partition-blocks of the hidden dim
JB = B // P  # 4 partition-blocks of the j axis
GS = 8  # output rows batched per store DMA

_cache_lock = threading.Lock()
_cached_nc = {}


def _build_bass(nloop=1):
    """Emit the Bass/Tile program for one core's [R, B] slab."""
    import concourse.bass as bass  # noqa: F401
    import concourse.tile as tile
    from concourse import bacc, mybir
    from concourse.masks import make_identity

    f32 = mybir.dt.float32
    f32r = mybir.dt.float32r
    Relu = mybir.ActivationFunctionType.Relu
    add = mybir.AluOpType.add
    amax = mybir.AluOpType.max

    nc = bacc.Bacc(
        "TRN2",
        target_bir_lowering=False,
        debug=False,
        enable_asserts=False,
    )

    x_d = nc.dram_tensor("x", (B, DX), f32, kind="ExternalInput").ap()
    ys_d = nc.dram_tensor("ys", (R, DY), f32, kind="ExternalInput").ap()
    w1_d = nc.dram_tensor("w1", (DX + DY, H), f32r, kind="ExternalInput").ap()
    b1_d = nc.dram_tensor("b1", (H,), f32, kind="ExternalInput").ap()
    w2_d = nc.dram_tensor("w2", (H, H), f32r, kind="ExternalInput").ap()
    b2_d = nc.dram_tensor("b2", (H,), f32, kind="ExternalInput").ap()
    w3_d = nc.dram_tensor("w3", (H, 1), f32r, kind="ExternalInput").ap()
    b3_d = nc.dram_tensor("b3", (1,), f32, kind="ExternalInput").ap()
    out_d = nc.dram_tensor("s_slab", (R, B), f32, kind="ExternalOutput").ap()

    def r32(ap):
        return ap.bitcast(f32r)

    with tile.TileContext(nc) as tc:
        with (
            tc.tile_pool(name="const", bufs=1) as cpool,
            tc.tile_pool(name="h1p", bufs=3) as h1pool,
            tc.tile_pool(name="h2p", bufs=3) as h2pool,
            tc.tile_pool(name="sgp", bufs=2) as spool,
            tc.tile_pool(name="ps_l2", bufs=4, space="PSUM") as ps_l2,
            tc.tile_pool(name="ps_aux", bufs=2, space="PSUM") as ps_aux,
        ):
            # ---------------- constants / weights ----------------
            ident = cpool.tile([P, P], f32)
            make_identity(nc, ident)

            w1x = cpool.tile([P, H], f32r)  # [dx, h]
            nc.sync.dma_start(w1x[:], w1_d[:DX, :])
            w1y = cpool.tile([P, H], f32r)  # [dy, h]
            nc.sync.dma_start(w1y[:], w1_d[DX:, :])
            w2 = cpool.tile([P, HB, H], f32r)  # [p, kb, m]: W2[kb*P+p, m]
            nc.sync.dma_start(w2[:], w2_d.rearrange("(kb p) m -> p kb m", p=P))
            w3 = cpool.tile([P, HB], f32r)  # W3[kb*P+p, 0]
            nc.sync.dma_start(w3[:], w3_d.rearrange("(kb p) m -> p (kb m)", p=P))
            b1 = cpool.tile([P, HB], f32)
            nc.sync.dma_start(b1[:], b1_d.rearrange("(o p) -> p o", p=P))
            b2 = cpool.tile([P, HB], f32)
            nc.sync.dma_start(b2[:], b2_d.rearrange("(o p) -> p o", p=P))
            b3 = cpool.tile([1, 1], f32)
            nc.sync.dma_start(b3[:], b3_d[None, :])

            # x natural layout, then PE-transpose to xT [dx, j]
            x_sb = cpool.tile([P, JB, DX], f32)  # x[jb*P+p, d]
            nc.sync.dma_start(x_sb[:], x_d.rearrange("(jb p) d -> p jb d", p=P))
            xT = cpool.tile([P, B], f32r)  # [dx, j]
            for jb in range(JB):
                ps_t = ps_aux.tile([P, P], f32, tag="tr")
                nc.tensor.transpose(ps_t[:], x_sb[:, jb, :], ident[:])
                nc.vector.tensor_copy(xT[:, jb * P : (jb + 1) * P], ps_t[:])

            ys_sb = cpool.tile([R, DY], f32)
            nc.sync.dma_start(ys_sb[:], ys_d[:, :])
            yT = cpool.tile([P, R], f32r)  # [dy, i]
            ps_t = ps_aux.tile([P, P], f32, tag="tr")
            nc.tensor.transpose(ps_t[:, :R], ys_sb[:], ident[:R, :R])
            nc.vector.tensor_copy(yT[:], ps_t[:, :R])

            # AT[h, j] = (x @ W1x).T ; CTb[h, i] = (ys @ W1y).T + b1[h]
            at = cpool.tile([P, HB, B], f32)
            ctb = cpool.tile([P, HB, R], f32)
            for hb in range(HB):
                hsl = slice(hb * P, (hb + 1) * P)
                ps_a = ps_l2.tile([P, B], f32, tag="l2")
                nc.tensor.matmul(ps_a[:], w1x[:, hsl], xT[:])
                nc.vector.tensor_copy(at[:, hb, :], ps_a[:])
                ps_c = ps_aux.tile([P, P], f32, tag="tr")
                nc.tensor.matmul(ps_c[:, :R], w1y[:, hsl], yT[:])
                nc.vector.tensor_scalar_add(
                    ctb[:, hb, :], ps_c[:, :R], scalar1=b1[:, hb : hb + 1]
                )

            # ---------------- main loop over the R y-rows ----------------
            # Layer-3 matmuls for row r are emitted during row r+1's layer-2
            # matmuls so the tensor engine never waits on the scalar engine.
            h2_live = {}
            sg_live = {}
            for it in range(nloop):
              for r in range(R + 1):
                if r < R:
                    # h1T = relu(AT + CTb[:, r])  (vector engine, one op/block)
                    h1 = h1pool.tile([P, HB, B], f32r, tag="h1")
                    for hb in range(HB):
                        nc.vector.tensor_scalar(
                            out=h1[:, hb, :],
                            in0=at[:, hb, :],
                            scalar1=ctb[:, hb, r : r + 1],
                            scalar2=0.0,
                            op0=add,
                            op1=amax,
                        )
                    # z2T = W2.T @ h1T ; h2T = relu(z2T + b2)
                    h2 = h2pool.tile([P, HB, B], f32r, tag="h2")
                    for mb in range(HB):
                        msl = slice(mb * P, (mb + 1) * P)
                        pl2 = ps_l2.tile([P, B], f32, tag="l2")
                        for kb in range(HB):
                            nc.tensor.matmul(
                                pl2[:],
                                w2[:, kb, msl],
                                h1[:, kb, :],
                                start=(kb == 0),
                                stop=(kb == HB - 1),
                            )
                        nc.scalar.activation(
                            h2[:, mb, :], pl2[:], Relu, bias=b2[:, mb : mb + 1]
                        )
                    h2_live[r] = h2

                rr = r - 1
                if rr >= 0:
                    # sT[rr, :] = W3.T @ h2T + b3  (M=1 matmuls)
                    h2p = h2_live.pop(rr)
                    ps_s = ps_aux.tile([1, B], f32, tag="s")
                    for kb in range(HB):
                        nc.tensor.matmul(
                            ps_s[:],
                            w3[:, kb : kb + 1],
                            h2p[:, kb, :],
                            start=(kb == 0),
                            stop=(kb == HB - 1),
                        )
                    g, gi = divmod(rr, GS)
                    if gi == 0:
                        sg_live[g] = spool.tile(
                            [1, GS, B], f32, tag="sg", name=f"sg_{it}_{g}"
                        )
                    nc.vector.tensor_scalar_add(
                        sg_live[g][:, gi, :], ps_s[:], scalar1=b3[:]
                    )
                    if gi == GS - 1:
                        sg = sg_live.pop(g)
                        nc.sync.dma_start(out_d[g * GS : (g + 1) * GS, :], sg[:])

    nc.compile()
    return nc


def _get_nc(nloop=1):
    with _cache_lock:
        if nloop not in _cached_nc:
            _cached_nc[nloop] = _build_bass(nloop)
        return _cached_nc[nloop]


def run(inputs, trace=False, **run_kwargs):
    """Shard, run on 8 cores, gather. Returns (out [B,B] f32, BassKernelResults)."""
    from concourse import bass_utils

    nc = _get_nc()
    x = np.ascontiguousarray(inputs["x"], dtype=np.float32)
    y = np.ascontiguousarray(inputs["y"], dtype=np.float32)
    common = {
        "x": x,
        "w1": np.ascontiguousarray(inputs["W1"], dtype=np.float32),
        "b1": np.ascontiguousarray(inputs["b1"], dtype=np.float32),
        "w2": np.ascontiguousarray(inputs["W2"], dtype=np.float32),
        "b2": np.ascontiguousarray(inputs["b2"], dtype=np.float32),
        "w3": np.ascontiguousarray(inputs["W3"], dtype=np.float32),
        "b3": np.ascontiguousarray(inputs["b3"], dtype=np.float32),
    }
    in_maps = [
        {**common, "ys": np.ascontiguousarray(y[d * R : (d + 1) * R])}
        for d in range(NCORES)
    ]
    res = bass_utils.run_bass_kernel_spmd(
        nc, in_maps, core_ids=list(range(NCORES)), trace=trace, **run_kwargs
    )
    s2 = np.concatenate([res.results[d]["s_slab"] for d in range(NCORES)], axis=0)
    return np.ascontiguousarray(s2.T), res


def kernel(**inputs) -> np.ndarray:
    # One retry: the axon-tunneled cores occasionally throw a transient
    # NRT_EXEC_UNIT_UNRECOVERABLE on the first touch after an idle period.
    try:
        out, _ = run(inputs, trace=False)
    except Exception:  # noqa: BLE001
        import time as _time

        _time.sleep(2.0)
        out, _ = run(inputs, trace=False)
    return out



# revision 10
# speedup vs baseline: 1.7666x; 1.7666x over previous
"""ConcatCritic all-pairs MLP scores on 8 Trainium2 NeuronCores.

scores[i, j] = MLP(concat(x[j], y[i])) computed as a [B, B] grid, sharded
by y-rows across 8 cores (each core computes a [B/8, B] slab).

Key restructure: layer 1 of the MLP acts on concat(x[j], y[i]), so
    z1[i, j, :] = x[j] @ W1x + (y[i] @ W1y + b1)
which is precomputed once as AT = (x @ W1x).T  [H, B] and
CT = (y_slab @ W1y + b1).T  [H, R].  Per row i, h1.T = relu(AT + CT[:, i])
is a single per-partition scalar add+max on the vector engine. This removes
the [B*B, 256] @ [256, 512] matmul entirely.

Layer 2 runs on the tensor engine in float32r (FP22 multiplies, FP32
accumulate) at 1 cycle/row -- the PE stream floor for this contraction.

Layer 3 no longer runs as M=1 matmuls (that cost 2048 PE cycles/row, 20%
of PE time). Instead the scalar engine emits h2 = relu(z2 + b2) in bf16,
the vector engine folds w3 in with a tensor_scalar + scalar_tensor_tensor
chain (g4 = sum_mb w3_mb * h2_mb, one [128, 512] bf16 block), and a single
ones-vector matmul (512 cycles) does the final 128-partition sum. b3 is
added on the host after the gather.
"""

import threading

import numpy as np

B = 512
DX = 128
DY = 128
H = 512
P = 128
NCORES = 8
R = B // NCORES  # 64 rows of the pair grid per core
HB = H // P  # 4 partition-blocks of the hidden dim
JB = B // P  # 4 partition-blocks of the j axis
GS = 8  # output rows batched per store DMA

_cache_lock = threading.Lock()
_cached_nc = {}


def _build_bass(nloop=1, trips=1):
    """Emit the Bass/Tile program for one core's [R, B] slab.

    nloop: python-unrolled repetitions of the 64-row body.
    trips: hardware-loop (For_i) repetitions around those; >1 is only used
    by the timing harness (total bodies = nloop * trips).
    """
    import concourse.bass as bass  # noqa: F401
    import concourse.tile as tile
    from concourse import bacc, mybir
    from concourse.masks import make_identity

    f32 = mybir.dt.float32
    f32r = mybir.dt.float32r
    bf16 = mybir.dt.bfloat16
    Relu = mybir.ActivationFunctionType.Relu
    Identity = mybir.ActivationFunctionType.Identity
    add = mybir.AluOpType.add
    amax = mybir.AluOpType.max
    mult = mybir.AluOpType.mult

    nc = bacc.Bacc(
        "TRN2",
        target_bir_lowering=False,
        debug=False,
        enable_asserts=False,
    )

    x_d = nc.dram_tensor("x", (B, DX), f32, kind="ExternalInput").ap()
    ys_d = nc.dram_tensor("ys", (R, DY), f32, kind="ExternalInput").ap()
    w1_d = nc.dram_tensor("w1", (DX + DY, H), f32r, kind="ExternalInput").ap()
    b1_d = nc.dram_tensor("b1", (H,), f32, kind="ExternalInput").ap()
    w2_d = nc.dram_tensor("w2", (H, H), f32r, kind="ExternalInput").ap()
    b2_d = nc.dram_tensor("b2", (H,), f32, kind="ExternalInput").ap()
    w3_d = nc.dram_tensor("w3", (H, 1), f32, kind="ExternalInput").ap()
    b3_d = nc.dram_tensor("b3", (1,), f32, kind="ExternalInput").ap()
    out_d = nc.dram_tensor("s_slab", (R, B), f32, kind="ExternalOutput").ap()

    with tile.TileContext(nc) as tc:
        with (
            tc.tile_pool(name="const", bufs=1) as cpool,
            tc.tile_pool(name="h1p", bufs=3) as h1pool,
            tc.tile_pool(name="h2p", bufs=3) as h2pool,
            tc.tile_pool(name="gp", bufs=2) as gpool,
            tc.tile_pool(name="sgp", bufs=2) as spool,
            tc.tile_pool(name="ps_l2", bufs=4, space="PSUM") as ps_l2,
            tc.tile_pool(name="ps_s", bufs=3, space="PSUM") as ps_spool,
            tc.tile_pool(name="ps_aux", bufs=1, space="PSUM") as ps_aux,
        ):
            # ---------------- constants / weights ----------------
            ident = cpool.tile([P, P], f32)
            make_identity(nc, ident)

            w1x = cpool.tile([P, H], f32r)  # [dx, h]
            nc.sync.dma_start(w1x[:], w1_d[:DX, :])
            w1y = cpool.tile([P, H], f32r)  # [dy, h]
            nc.sync.dma_start(w1y[:], w1_d[DX:, :])
            w2 = cpool.tile([P, HB, H], f32r)  # [p, kb, m]: W2[kb*P+p, m]
            nc.sync.dma_start(w2[:], w2_d.rearrange("(kb p) m -> p kb m", p=P))
            w3 = cpool.tile([P, HB], f32)  # W3[mb*P+p, 0]
            nc.sync.dma_start(w3[:], w3_d.rearrange("(kb p) m -> p (kb m)", p=P))
            b1 = cpool.tile([P, HB], f32)
            nc.sync.dma_start(b1[:], b1_d.rearrange("(o p) -> p o", p=P))
            b2 = cpool.tile([P, HB], f32)
            nc.sync.dma_start(b2[:], b2_d.rearrange("(o p) -> p o", p=P))
            b3 = cpool.tile([1, 1], f32)
            nc.sync.dma_start(b3[:], b3_d[None, :])
            ones_bf = cpool.tile([P, 1], bf16)
            nc.vector.memset(ones_bf[:], 1.0)

            # x natural layout, then PE-transpose to xT [dx, j]
            x_sb = cpool.tile([P, JB, DX], f32)  # x[jb*P+p, d]
            nc.sync.dma_start(x_sb[:], x_d.rearrange("(jb p) d -> p jb d", p=P))
            xT = cpool.tile([P, B], f32r)  # [dx, j]
            for jb in range(JB):
                ps_t = ps_aux.tile([P, P], f32, tag="tr")
                nc.tensor.transpose(ps_t[:], x_sb[:, jb, :], ident[:])
                nc.vector.tensor_copy(xT[:, jb * P : (jb + 1) * P], ps_t[:])

            ys_sb = cpool.tile([R, DY], f32)
            nc.sync.dma_start(ys_sb[:], ys_d[:, :])
            yT = cpool.tile([P, R], f32r)  # [dy, i]
            ps_t = ps_aux.tile([P, P], f32, tag="tr")
            nc.tensor.transpose(ps_t[:, :R], ys_sb[:], ident[:R, :R])
            nc.vector.tensor_copy(yT[:], ps_t[:, :R])

            # AT[h, j] = (x @ W1x).T ; CTb[h, i] = (ys @ W1y).T + b1[h]
            at = cpool.tile([P, HB, B], f32)
            ctb = cpool.tile([P, HB, R], f32)
            for hb in range(HB):
                hsl = slice(hb * P, (hb + 1) * P)
                ps_a = ps_l2.tile([P, B], f32, tag="l2")
                nc.tensor.matmul(ps_a[:], w1x[:, hsl], xT[:])
                nc.vector.tensor_copy(at[:, hb, :], ps_a[:])
                ps_c = ps_aux.tile([P, P], f32, tag="tr")
                nc.tensor.matmul(ps_c[:, :R], w1y[:, hsl], yT[:])
                nc.vector.tensor_scalar_add(
                    ctb[:, hb, :], ps_c[:, :R], scalar1=b1[:, hb : hb + 1]
                )

            # ---------------- main loop over the R y-rows ----------------
            # Per row r: DVE h1 -> PE layer2 (16 MMs) -> ACT h2 (bf16)
            #  -> DVE w3-fold chain -> PE ones-reduction -> DMA out.
            # Row r's tail work is emitted during row r+1 so the tensor
            # engine stream never waits on the scalar/vector engines.
            def body(it):
                h2_live = {}
                sg_live = {}
                for r in range(R + 1):
                    if r < R:
                        # h1T = relu(AT + CTb[:, r])  (vector engine)
                        h1 = h1pool.tile([P, HB, B], f32r, tag="h1")
                        for hb in range(HB):
                            nc.vector.tensor_scalar(
                                out=h1[:, hb, :],
                                in0=at[:, hb, :],
                                scalar1=ctb[:, hb, r : r + 1],
                                scalar2=0.0,
                                op0=add,
                                op1=amax,
                            )
                        # z2T = W2.T @ h1T ; h2T = relu(z2T + b2) in bf16
                        h2 = h2pool.tile([P, HB, B], bf16, tag="h2")
                        for mb in range(HB):
                            msl = slice(mb * P, (mb + 1) * P)
                            pl2 = ps_l2.tile([P, B], f32, tag="l2")
                            for kb in range(HB):
                                nc.tensor.matmul(
                                    pl2[:],
                                    w2[:, kb, msl],
                                    h1[:, kb, :],
                                    start=(kb == 0),
                                    stop=(kb == HB - 1),
                                )
                            nc.scalar.activation(
                                h2[:, mb, :], pl2[:], Relu, bias=b2[:, mb : mb + 1]
                            )
                        h2_live[r] = h2

                    rr = r - 1
                    if rr >= 0:
                        # g4 = sum_mb w3_mb * h2_mb  (DVE, bf16)
                        h2p = h2_live.pop(rr)
                        acc = gpool.tile([P, B], bf16, tag="g0")
                        nc.vector.tensor_scalar_mul(
                            acc, h2p[:, 0, :], scalar1=w3[:, 0:1]
                        )
                        for mb in range(1, HB):
                            acc2 = gpool.tile([P, B], bf16, tag=f"g{mb}")
                            nc.vector.scalar_tensor_tensor(
                                out=acc2,
                                in0=h2p[:, mb, :],
                                scalar=w3[:, mb : mb + 1],
                                in1=acc,
                                op0=mult,
                                op1=add,
                            )
                            acc = acc2
                        # sT[rr, :] = ones.T @ g4  (single 512-cycle matmul)
                        ps_s = ps_spool.tile([1, B], f32, tag="s")
                        nc.tensor.matmul(
                            ps_s[:], ones_bf[:], acc, start=True, stop=True
                        )
                        # + b3, PSUM -> SBUF on the scalar engine; DMA per GS
                        g, gi = divmod(rr, GS)
                        if gi == 0:
                            sg_live[g] = spool.tile(
                                [1, GS, B], f32, tag="sg", name=f"sg_{it}_{g}"
                            )
                        nc.scalar.activation(
                            sg_live[g][:, gi, :], ps_s[:], Identity, bias=b3[:]
                        )
                        if gi == GS - 1:
                            sg = sg_live.pop(g)
                            nc.sync.dma_start(out_d[g * GS : (g + 1) * GS, :], sg[:])

            if trips > 1:
                with tc.For_i(0, trips):
                    for it in range(nloop):
                        body(it)
            else:
                for it in range(nloop):
                    body(it)

    nc.compile()
    return nc


def _get_nc(nloop=1, trips=1):
    with _cache_lock:
        key = (nloop, trips)
        if key not in _cached_nc:
            _cached_nc[key] = _build_bass(nloop, trips)
        return _cached_nc[key]


def run(inputs, trace=False, **run_kwargs):
    """Shard, run on 8 cores, gather. Returns (out [B,B] f32, BassKernelResults)."""
    from concourse import bass_utils

    nc = _get_nc()
    x = np.ascontiguousarray(inputs["x"], dtype=np.float32)
    y = np.ascontiguousarray(inputs["y"], dtype=np.float32)
    common = {
        "x": x,
        "w1": np.ascontiguousarray(inputs["W1"], dtype=np.float32),
        "b1": np.ascontiguousarray(inputs["b1"], dtype=np.float32),
        "w2": np.ascontiguousarray(inputs["W2"], dtype=np.float32),
        "b2": np.ascontiguousarray(inputs["b2"], dtype=np.float32),
        "w3": np.ascontiguousarray(inputs["W3"], dtype=np.float32),
        "b3": np.ascontiguousarray(inputs["b3"], dtype=np.float32),
    }
    in_maps = [
        {**common, "ys": np.ascontiguousarray(y[d * R : (d + 1) * R])}
        for d in range(NCORES)
    ]
    res = bass_utils.run_bass_kernel_spmd(
        nc, in_maps, core_ids=list(range(NCORES)), trace=trace, **run_kwargs
    )
    s2 = np.concatenate([res.results[d]["s_slab"] for d in range(NCORES)], axis=0)
    return np.ascontiguousarray(s2.T), res


def kernel(**inputs) -> np.ndarray:
    # One retry: the axon-tunneled cores occasionally throw a transient
    # NRT_EXEC_UNIT_UNRECOVERABLE on the first touch after an idle period.
    try:
        out, _ = run(inputs, trace=False)
    except Exception:  # noqa: BLE001
        import time as _time

        _time.sleep(2.0)
        out, _ = run(inputs, trace=False)
    return out


# revision 14
# speedup vs baseline: 1.8265x; 1.0339x over previous
"""ConcatCritic all-pairs MLP scores on 8 Trainium2 NeuronCores.

scores[i, j] = MLP(concat(x[j], y[i])) computed as a [B, B] grid, sharded
by y-rows across 8 cores (each core computes a [B/8, B] slab).

Structure (per core, per grid row r):
  - layer 1 is precomputed: AT = (x @ W1x).T [H, B] once, CT = (ys @ W1y
    + b1).T [H, R] once; h1 = relu(AT + CT[:, r]) is 4 vector-engine
    tensor_scalar ops (fp32 in, bf16 out).
  - layer 2 runs on the tensor engine in bf16 (16 matmuls of
    [128,128]x[128,512] into 4 PSUM banks). W2 is pre-scaled on the host
    by |w3| per column and permuted (see below), so layer 3's weight
    multiply costs nothing.
  - layer 3: the scalar engine applies relu+bias per bank (bf16 out,
    giving g = |w3| * h2), the vector engine folds the 4 hidden blocks
    with one [128,1024] tensor_tensor add, and two accumulating M=1
    matmuls with a +-1 stationary vector v finish the 512-way hidden sum
    in 2x512 PE cycles (vs 4x512 for naive M=1 layer-3 matmuls).

The sign trick: hidden units are permuted on the host so that each
partition row p holds 4 units (one per block) of equal sign(w3); the
per-partition +-1 vector v then applies the sign after the relu. When
the positive count isn't divisible by 4, the <=3 smallest-|w3| units are
flipped to the other side (error ~1e-3 relative, verified in sim).

Outputs batch 4 rows per PSUM bank via matmul col-group placement
(partitions 0/32/64/96), one DVE copy moves them to SBUF, and the DMA
picks the 4 live partitions. b3 is added on the host after the gather.
"""

import threading

import numpy as np

B = 512
DX = 128
DY = 128
H = 512
P = 128
NCORES = 8
R = B // NCORES  # 64 rows of the pair grid per core
HB = H // P  # 4 partition-blocks of the hidden dim
JB = B // P  # 4 partition-blocks of the j axis

_cache_lock = threading.Lock()
_cached_nc = {}


def _build_bass(nloop=1, trips=1):
    """Emit the Bass/Tile program for one core's [R, B] slab.

    nloop: python-unrolled repetitions of the 64-row body.
    trips: hardware-loop (For_i) repetitions around those; >1 is only used
    by the timing harness (total bodies = nloop * trips).
    """
    import concourse.bass as bass  # noqa: F401
    import concourse.tile as tile
    from concourse import bacc, mybir
    from concourse.masks import make_identity

    f32 = mybir.dt.float32
    f32r = mybir.dt.float32r
    bf16 = mybir.dt.bfloat16
    Relu = mybir.ActivationFunctionType.Relu
    add = mybir.AluOpType.add
    amax = mybir.AluOpType.max

    nc = bacc.Bacc(
        "TRN2",
        target_bir_lowering=False,
        debug=False,
        enable_asserts=False,
    )

    x_d = nc.dram_tensor("x", (B, DX), f32, kind="ExternalInput").ap()
    ys_d = nc.dram_tensor("ys", (R, DY), f32, kind="ExternalInput").ap()
    w1_d = nc.dram_tensor("w1", (DX + DY, H), f32r, kind="ExternalInput").ap()
    b1_d = nc.dram_tensor("b1", (H,), f32, kind="ExternalInput").ap()
    w2s_d = nc.dram_tensor("w2s", (H, H), bf16, kind="ExternalInput").ap()
    b2s_d = nc.dram_tensor("b2s", (H,), f32, kind="ExternalInput").ap()
    v_d = nc.dram_tensor("v", (P, 1), bf16, kind="ExternalInput").ap()
    out_d = nc.dram_tensor("s_slab", (R, B), f32, kind="ExternalOutput").ap()

    with tile.TileContext(nc) as tc:
        with (
            tc.tile_pool(name="const", bufs=1) as cpool,
            tc.tile_pool(name="h1p", bufs=3) as h1pool,
            tc.tile_pool(name="h2p", bufs=2) as h2pool,
            tc.tile_pool(name="tp", bufs=2) as tpool,
            tc.tile_pool(name="sgp", bufs=2) as spool,
            tc.tile_pool(name="ps_l2", bufs=4, space="PSUM") as ps_l2,
            tc.tile_pool(name="ps_s", bufs=2, space="PSUM") as ps_spool,
            tc.tile_pool(name="ps_aux", bufs=1, space="PSUM") as ps_aux,
        ):
            # ---------------- constants / weights ----------------
            ident = cpool.tile([P, P], f32)
            make_identity(nc, ident)

            w1x = cpool.tile([P, H], f32r)  # [dx, h]
            nc.sync.dma_start(w1x[:], w1_d[:DX, :])
            w1y = cpool.tile([P, H], f32r)  # [dy, h]
            nc.sync.dma_start(w1y[:], w1_d[DX:, :])
            w2 = cpool.tile([P, HB, H], bf16)  # [p, kb, m]: W2s[kb*P+p, m]
            nc.sync.dma_start(w2[:], w2s_d.rearrange("(kb p) m -> p kb m", p=P))
            b1 = cpool.tile([P, HB], f32)
            nc.sync.dma_start(b1[:], b1_d.rearrange("(o p) -> p o", p=P))
            b2s = cpool.tile([P, HB], f32)
            nc.sync.dma_start(b2s[:], b2s_d.rearrange("(o p) -> p o", p=P))
            v = cpool.tile([P, 1], bf16)
            nc.sync.dma_start(v[:], v_d[:, :])

            # x natural layout, then PE-transpose to xT [dx, j]
            x_sb = cpool.tile([P, JB, DX], f32)  # x[jb*P+p, d]
            nc.sync.dma_start(x_sb[:], x_d.rearrange("(jb p) d -> p jb d", p=P))
            xT = cpool.tile([P, B], f32r)  # [dx, j]
            for jb in range(JB):
                ps_t = ps_aux.tile([P, P], f32, tag="tr")
                nc.tensor.transpose(ps_t[:], x_sb[:, jb, :], ident[:])
                nc.vector.tensor_copy(xT[:, jb * P : (jb + 1) * P], ps_t[:])

            ys_sb = cpool.tile([R, DY], f32)
            nc.sync.dma_start(ys_sb[:], ys_d[:, :])
            yT = cpool.tile([P, R], f32r)  # [dy, i]
            ps_t = ps_aux.tile([P, P], f32, tag="tr")
            nc.tensor.transpose(ps_t[:, :R], ys_sb[:], ident[:R, :R])
            nc.vector.tensor_copy(yT[:], ps_t[:, :R])

            # AT[h, j] = (x @ W1x).T ; CTb[h, i] = (ys @ W1y).T + b1[h]
            at = cpool.tile([P, HB, B], f32)
            ctb = cpool.tile([P, HB, R], f32)
            for hb in range(HB):
                hsl = slice(hb * P, (hb + 1) * P)
                ps_a = ps_l2.tile([P, B], f32, tag="l2")
                nc.tensor.matmul(ps_a[:], w1x[:, hsl], xT[:])
                nc.vector.tensor_copy(at[:, hb, :], ps_a[:])
                ps_c = ps_aux.tile([P, P], f32, tag="tr")
                nc.tensor.matmul(ps_c[:, :R], w1y[:, hsl], yT[:])
                nc.vector.tensor_scalar_add(
                    ctb[:, hb, :], ps_c[:, :R], scalar1=b1[:, hb : hb + 1]
                )

            # ---------------- main loop over the R y-rows ----------------
            # Software pipeline (iteration k): h1 leads one row, the g-fold
            # tree lags one, the sign matmuls lag two, output copies drain
            # per 4 rows. The PE stream (16 L2 matmuls + 2 M=1 sign MMs per
            # row) never waits on the scalar/vector engines.
            def emit_h1(r):
                h1 = h1pool.tile([P, HB, B], bf16, tag="h1", name=f"h1_{r}")
                for hb in range(HB):
                    nc.vector.tensor_scalar(
                        out=h1[:, hb, :],
                        in0=at[:, hb, :],
                        scalar1=ctb[:, hb, r : r + 1],
                        scalar2=0.0,
                        op0=add,
                        op1=amax,
                    )
                return h1

            def body(it):
                h1_live = {}
                h2_live = {}
                t_live = {}
                ps_live = {}
                sg_live = {}
                h1_live[0] = emit_h1(0)
                for k in range(R + 2):
                    # sign matmuls for row k-2 (2 accumulating M=1 MMs).
                    # Col-group placement batches 3 rows per PSUM bank at
                    # partitions {0,32,64} (col-group 3 is unusable: HW bug).
                    rr = k - 2
                    if rr >= 0:
                        g, gi = divmod(rr, 3)
                        if gi == 0:
                            ps_live[g] = ps_spool.tile(
                                [65, B], f32, tag="s", name=f"ps_{it}_{g}"
                            )
                        tprev = t_live.pop(rr)
                        slot = ps_live[g][32 * gi : 32 * gi + 1, :]
                        nc.tensor.matmul(
                            slot, v[:], tprev[:, 0, :], start=True, stop=False
                        )
                        nc.tensor.matmul(
                            slot, v[:], tprev[:, 1, :], start=False, stop=True
                        )
                    # h1 for row k+1 (DVE ahead of the PE stream)
                    if k + 1 <= R - 1:
                        h1_live[k + 1] = emit_h1(k + 1)
                    # layer-2 matmuls + relu for row k
                    if k <= R - 1:
                        h1 = h1_live.pop(k)
                        h2 = h2pool.tile([P, HB, B], bf16, tag="h2")
                        for mb in range(HB):
                            msl = slice(mb * P, (mb + 1) * P)
                            pl2 = ps_l2.tile([P, B], f32, tag="l2")
                            for kb in range(HB):
                                nc.tensor.matmul(
                                    pl2[:],
                                    w2[:, kb, msl],
                                    h1[:, kb, :],
                                    start=(kb == 0),
                                    stop=(kb == HB - 1),
                                )
                            nc.scalar.activation(
                                h2[:, mb, :], pl2[:], Relu, bias=b2s[:, mb : mb + 1]
                            )
                        h2_live[k] = h2
                    # block-fold for row k-1: t = (g0+g2, g1+g3), one TT op
                    rt = k - 1
                    if 0 <= rt <= R - 1:
                        h2p = h2_live.pop(rt)
                        t = tpool.tile([P, 2, B], bf16, tag="t")
                        nc.vector.tensor_tensor(
                            out=t[:],
                            in0=h2p[:, 0:2, :],
                            in1=h2p[:, 2:4, :],
                            op=add,
                        )
                        t_live[rt] = t
                    # drain a full group of 3 output rows (copy + DMA)
                    if rr >= 0 and (rr % 3 == 2 or rr == R - 1):
                        g = rr // 3
                        nrows = rr % 3 + 1
                        psb = ps_live.pop(g)
                        sg = spool.tile([65, B], f32, tag="sg", name=f"sg_{it}_{g}")
                        nc.vector.tensor_copy(sg[:], psb[:])
                        sg_live[g] = sg
                        nc.sync.dma_start(
                            out_d[3 * g : 3 * g + nrows, :],
                            sg[0 : 32 * nrows - 31 : 32, :],
                        )
                        sg_live.pop(g)

            if trips > 1:
                with tc.For_i(0, trips):
                    for it in range(nloop):
                        body(it)
            else:
                for it in range(nloop):
                    body(it)

    nc.compile()
    return nc


def _get_nc(nloop=1, trips=1):
    with _cache_lock:
        key = (nloop, trips)
        if key not in _cached_nc:
            _cached_nc[key] = _build_bass(nloop, trips)
        return _cached_nc[key]


def make_in_maps(inputs):
    """Host-side prep: sign permutation of hidden units, |w3| pre-scale of
    W2's columns, bf16 weight cast, per-core y sharding."""
    import ml_dtypes

    x = np.ascontiguousarray(inputs["x"], dtype=np.float32)
    y = np.ascontiguousarray(inputs["y"], dtype=np.float32)
    W2 = np.asarray(inputs["W2"], dtype=np.float32)
    b2 = np.asarray(inputs["b2"], dtype=np.float32).reshape(-1)
    w3 = np.asarray(inputs["W3"], dtype=np.float32).reshape(-1)

    sgn = np.where(w3 > 0, 1.0, -1.0).astype(np.float32)
    t = int((sgn > 0).sum()) % 4
    if t:
        neg_idx = np.where(sgn < 0)[0]
        pos_idx = np.where(sgn > 0)[0]
        neg_small = neg_idx[np.argsort(np.abs(w3[neg_idx]))[: 4 - t]]
        pos_small = pos_idx[np.argsort(np.abs(w3[pos_idx]))[:t]]
        if np.abs(w3[neg_small]).sum() <= np.abs(w3[pos_small]).sum():
            sgn[neg_small] = 1.0
        else:
            sgn[pos_small] = -1.0
    order = np.argsort(-sgn, kind="stable")
    c = np.arange(H)
    new_cols = order[4 * (c % P) + c // P]
    w3a = np.abs(w3[new_cols])
    W2s = np.ascontiguousarray(
        (W2[:, new_cols] * w3a[None, :]).astype(ml_dtypes.bfloat16)
    )
    b2s = np.ascontiguousarray(b2[new_cols] * w3a)
    vcol = np.ascontiguousarray(
        sgn[new_cols][:P].reshape(P, 1).astype(ml_dtypes.bfloat16)
    )

    common = {
        "x": x,
        "w1": np.ascontiguousarray(inputs["W1"], dtype=np.float32),
        "b1": np.ascontiguousarray(inputs["b1"], dtype=np.float32),
        "w2s": W2s,
        "b2s": b2s,
        "v": vcol,
    }
    return [
        {**common, "ys": np.ascontiguousarray(y[d * R : (d + 1) * R])}
        for d in range(NCORES)
    ]


def run(inputs, trace=False, **run_kwargs):
    """Shard, run on 8 cores, gather. Returns (out [B,B] f32, BassKernelResults)."""
    from concourse import bass_utils

    nc = _get_nc()
    in_maps = make_in_maps(inputs)
    res = bass_utils.run_bass_kernel_spmd(
        nc, in_maps, core_ids=list(range(NCORES)), trace=trace, **run_kwargs
    )
    s2 = np.concatenate([res.results[d]["s_slab"] for d in range(NCORES)], axis=0)
    b3 = np.float32(np.asarray(inputs["b3"]).reshape(-1)[0])
    return np.ascontiguousarray(s2.T + b3), res


def kernel(**inputs) -> np.ndarray:
    # One retry: the axon-tunneled cores occasionally throw a transient
    # NRT_EXEC_UNIT_UNRECOVERABLE on the first touch after an idle period.
    try:
        out, _ = run(inputs, trace=False)
    except Exception:  # noqa: BLE001
        import time as _time

        _time.sleep(2.0)
        out, _ = run(inputs, trace=False)
    return out
